# revision 1
# baseline (speedup 1.0000x reference)
"""Trainium2 Bass kernel for nn_HausdorffDistance_28406913696124.

Math (reference):
    px = (prob_map[0].ravel() >= 0.5)                 # [N], N = 100*100
    py = (gt_map.ravel()   >= 0.5)                    # [N]
    D[i,j] = euclid dist between grid points i, j     # [N, N] constant!
    loss   = mean_i | px_i * mean_j D[i,j] - (D @ py)_i / N |

Key structure: D depends only on (|r_i-r_j|, |c_i-c_j|) with r=i//100,
c=i%100.  So:
  * rowmean_i = mean_j D[i,j] is a pure constant -> precomputed on host.
  * (D @ py) is a 2D correlation of the 100x100 binary mask PY with the
    199x199 kernel sqrt(dr^2+dc^2).  Factor it through the distance table
    Q[u,v] = sqrt(u^2+v^2), u,v in [0,100):

        term2sum[r,c] = sum_d ( H_d[r-d, c] + H_d[r+d, c] )   (d=0 once)
        H_d  = PY @ T_d          T_d[b,c] = Q[d, |b-c|]   (Toeplitz)

    On the PE the +-d row shifts are folded into the stationary operand:
    for the 13 d's owned by a core (d = 13k + j, j = 0..12) the stationary
    C_j[b, r] = PYT_plus[b, r-j] + PYT_minus[b, r+j], where PYT_plus/minus
    are the transposed binary mask pre-shifted by +-13k (shift baked into
    the per-core input data, so the SPMD program only uses j = 0..12 as
    compile-time AP offsets).  One strided DVE add builds all 13 C_j from
    zero-padded tiles; 13 accumulating matmuls then produce this core's
    partial term2sum [100,100] directly in PSUM.

Sharding: 100 d-values split across 8 cores (13/core, zero padded), an
on-device AllReduce sums the partial term2sum maps, then every core
computes the identical final scalar.
"""

import sys

import numpy as np

sys.path.insert(0, "/opt/trn_rl_repo")

H = 100
N = H * H
NCORES = 8
DSH = 13   # d-values per core (8*13 = 104 >= 100, rest zero-padded)
PADW = 13  # zero pad on each side of the transposed-mask tiles
CHUNK = 500  # free-dim chunk (matmul <= 512 fp32 PSUM bank)


def _host_constants():
    """Geometry-only constant tables (input independent)."""
    idx = np.arange(H)
    absdiff = np.abs(idx[:, None] - idx[None, :])  # [100,100] |b-c|
    # fp32-exact integer squares -> correctly rounded fp32 sqrt: matches the
    # reference's gram-matrix + sqrt exactly.
    q32 = np.sqrt((idx[:, None] ** 2 + idx[None, :] ** 2).astype(np.float32))

    # rowsum[r,c] = sum_j D[i,j] (i = r*100+c), accumulated in float64.
    # (N * rowmean -- the 1/N^2 is folded into the final scalar scale.)
    cnt = np.zeros((H, H))  # cnt[r,u] = #{a : |r-a| = u}
    np.add.at(cnt, (idx[:, None], absdiff), 1.0)
    # negated so the device can fold "- px*rowsumN" into the 9-way
    # gather-sum (see _build_module).
    rowsumN = (-(cnt @ q32.astype(np.float64) @ cnt.T)).astype(np.float32)

    q16 = q32.astype(np.float16)
    t_slices = []
    for k in range(NCORES):
        t_k = np.zeros((H, DSH * H), dtype=np.float16)
        for j in range(DSH):
            d = k * DSH + j
            if d >= H:
                continue
            blk = q16[d, absdiff]
            if d == 0:
                # d=0 appears in both the +j and -j branch of the combined
                # stationary; halve once so it is counted once.
                blk = (blk.astype(np.float32) * 0.5).astype(np.float16)
            t_k[:, j * H:(j + 1) * H] = blk
        t_slices.append(t_k)
    return rowsumN, t_slices


def _build_module(with_collective=True):
    import concourse.bacc as bacc
    import concourse.mybir as mybir
    import concourse.tile as tile

    f32 = mybir.dt.float32
    f16 = mybir.dt.float16

    nc = bacc.Bacc(
        "TRN2",
        target_bir_lowering=False,
        debug=False,
        enable_asserts=False,
        num_devices=NCORES,
    )

    # gtpack = gtT_plus | gtT_minus ; rmprob = rowsumN | prob  ([100,100] f32)
    gtpack_d = nc.dram_tensor("gtpack", [H, 2 * H], f32, kind="ExternalInput")
    rmprob_d = nc.dram_tensor("rmprob", [H, 2 * H], f32, kind="ExternalInput")
    tsl_d = nc.dram_tensor("t_slice", [H, DSH * H], f16, kind="ExternalInput")
    out_d = nc.dram_tensor("out", [1, 1], f32, kind="ExternalOutput")

    PW = H + 2 * PADW  # padded width of the transposed-mask tiles

    with tile.TileContext(nc) as tc:
        with (
            tc.tile_pool(name="sb", bufs=1) as sb,
            tc.tile_pool(name="ps_acc", bufs=1, space="PSUM") as ps_acc,
            tc.tile_pool(name="ps_fin", bufs=1, space="PSUM") as ps_fin,
            tc.tile_pool(name="dram", bufs=1, space="DRAM") as dram,
        ):
            # ---- loads (gt/rm on ACT ring, T on SP ring; a single
            # InstDMACopy is split across all 16 SDMA engines on HW) ------
            gtpack_sb = sb.tile([H, 2 * H], f32)
            nc.scalar.dma_start(gtpack_sb[:], gtpack_d[:])
            gtp_sb = gtpack_sb[:, 0:H]
            gtm_sb = gtpack_sb[:, H:2 * H]
            rmprob_sb = sb.tile([H, 2 * H], f32)
            nc.scalar.dma_start(rmprob_sb[:], rmprob_d[:])
            rm_sb = rmprob_sb[:, 0:H]
            prob_sb = rmprob_sb[:, H:2 * H]

            tsl_sb = sb.tile([H, DSH * H], f16)
            nc.sync.dma_start(tsl_sb[:], tsl_d[:])

            # ---- binarize the pre-shifted transposed masks -------------
            pytp = sb.tile([H, PW], f16)  # PYT_plus, zero padded
            pytm = sb.tile([H, PW], f16)  # PYT_minus, zero padded
            nc.vector.memset(pytp[:], 0.0)
            nc.vector.memset(pytm[:], 0.0)
            nc.vector.tensor_scalar(
                pytp[:, PADW:PADW + H], gtp_sb, 0.5, None, mybir.AluOpType.is_ge
            )
            nc.vector.tensor_scalar(
                pytm[:, PADW:PADW + H], gtm_sb, 0.5, None, mybir.AluOpType.is_ge
            )

            # ---- combined stationary: C_j[b, m] = pytp[b, PADW-j+m]
            #                                     + pytm[b, PADW+j+m] ----
            comb = sb.tile([H, DSH * H], f16)
            for j in range(DSH):
                nc.vector.tensor_add(
                    comb[:, j * H:(j + 1) * H],
                    pytp[:, PADW - j:PADW - j + H],
                    pytm[:, PADW + j:PADW + j + H],
                )

            # ---- 13 accumulating matmuls -> partial term2sum in PSUM ---
            acc_ps = ps_acc.tile([H, H], f32)
            for j in range(DSH):
                nc.tensor.matmul(
                    acc_ps[:],
                    comb[:, j * H:(j + 1) * H],
                    tsl_sb[:, j * H:(j + 1) * H],
                    start=(j == 0),
                    stop=(j == DSH - 1),
                )
            # ---- AllGather the 8 partial maps, sum them on-device ------
            # (AG floor ~5us vs AR ~10us on 8 cores; the 8-way sum is one
            # strided DVE reduce over a [100, 100, 8] view.)
            part2 = sb.tile([H, H], f32)
            nc.vector.tensor_copy(part2[:], acc_ps[:])
            cc_in = dram.tile([H, H], f32)
            cc_out = dram.tile([NCORES * H, H], f32, addr_space="Shared")
            nc.sync.dma_start(cc_in[:], part2[:])
            if with_collective:
                nc.gpsimd.collective_compute(
                    "AllGather",
                    mybir.AluOpType.bypass,
                    replica_groups=[list(range(NCORES))],
                    ins=[cc_in[:].opt()],
                    outs=[cc_out[:].opt()],
                )
                gath_src = cc_out[:]
            else:
                # timing-model variant (no collectives in sim): fake the
                # gather with a single same-sized DRAM read.
                gath_src = cc_out[:]
            # gath slices g=0..7: the gathered partial maps; slice 8:
            # t1n = px * (-rowsumN).  One strided 9-way reduce then gives
            # diff = term2sum - px*rowsumN directly.
            gath = sb.tile([H, (NCORES + 1) * H], f32)
            nc.vector.scalar_tensor_tensor(
                gath[:, NCORES * H:(NCORES + 1) * H],
                prob_sb,
                0.5,
                rm_sb,
                op0=mybir.AluOpType.is_ge,
                op1=mybir.AluOpType.mult,
            )
            # DRAM [g*H + p, c] -> SBUF [p, g*H + c]
            nc.scalar.dma_start(
                gath[:, 0:NCORES * H].rearrange("p (g c) -> p g c", g=NCORES),
                gath_src.rearrange("(g p) c -> p g c", g=NCORES),
            )
            diff = sb.tile([H, H], f32)
            nc.vector.tensor_reduce(
                diff[:],
                gath[:].rearrange("p (g c) -> p c g", g=NCORES + 1),
                axis=mybir.AxisListType.X,
                op=mybir.AluOpType.add,
            )
            rowsums = sb.tile([H, 1], f32)
            nc.vector.tensor_reduce(
                rowsums[:],
                diff[:],
                axis=mybir.AxisListType.X,
                op=mybir.AluOpType.add,
                apply_absolute_value=True,
            )
            ones_sb = sb.tile([H, 1], f32)
            nc.vector.memset(ones_sb[:], 1.0)
            fin_ps = ps_fin.tile([1, 1], f32)
            nc.tensor.matmul(fin_ps[:], rowsums[:], ones_sb[:])
            out_sb = sb.tile([1, 1], f32)
            nc.vector.tensor_scalar_mul(out_sb[:], fin_ps[:], 1.0 / (N * N))
            nc.sync.dma_start(out_d[:], out_sb[:])

    nc.compile()
    return nc


_STATE = {}


def _get_state():
    if not _STATE:
        rowsumN, t_slices = _host_constants()
        _STATE["consts"] = (rowsumN, t_slices)
        _STATE["nc"] = _build_module()
    return _STATE


def _in_maps(prob_map, gt_map):
    st = _get_state()
    rowsumN, t_slices = st["consts"]
    prob = np.asarray(prob_map, dtype=np.float32).reshape(H, H)
    gt = np.asarray(gt_map, dtype=np.float32).reshape(H, H)
    gtT = np.ascontiguousarray(gt.T)

    rmprob = np.ascontiguousarray(np.concatenate([rowsumN, prob], axis=1))
    in_maps = []
    for k in range(NCORES):
        dk = k * DSH
        gtp = np.zeros((H, H), dtype=np.float32)
        gtm = np.zeros((H, H), dtype=np.float32)
        gtp[:, dk:] = gtT[:, :H - dk]
        gtm[:, :H - dk] = gtT[:, dk:]
        gtpack = np.ascontiguousarray(np.concatenate([gtp, gtm], axis=1))
        in_maps.append(
            {"gtpack": gtpack, "rmprob": rmprob, "t_slice": t_slices[k]}
        )
    return in_maps


def _run(prob_map, gt_map, trace=False, **spmd_kwargs):
    from concourse import bass_utils

    st = _get_state()
    in_maps = _in_maps(prob_map, gt_map)
    res = bass_utils.run_bass_kernel_spmd(
        st["nc"], in_maps, core_ids=list(range(NCORES)), trace=trace,
        **spmd_kwargs,
    )
    value = np.float32(res.results[0]["out"][0, 0])
    return value, res


def kernel(prob_map, gt_map):
    value, _ = _run(prob_map, gt_map, trace=False)
    return np.asarray(value, dtype=np.float32)

